# revision 31
# baseline (speedup 1.0000x reference)
"""Batch-Jacobian of a 3-layer tanh MLP (64->256->256->64), B=8192.

J[b] = W3^T diag(1-h2^2) W2^T diag(1-h1^2) W1^T   (shape 64x64 per b)

U-path strategy (per core, 1024 batch elems, windows of 512):
  forward -> d1[k,b], d2[m,b] (bf16); window 1's forward is deferred into the
  pair loop so it overlaps window 0's jacobian work.
  Precompute U_i[k,m] = W1[i,k]*W2[k,m] host-side (8 MB bf16, stationary),
  streamed in 8-i chunks interleaved across the 4 (kh,mh) tensors so the
  first pairs unblock ~6us in.
  Pair loop (32 pairs x 4 i-values: even half -> psum partitions 0-63, odd
  half -> 64-127 via matmul tile_position), software-pipelined so PE's C2
  matmuls of pair n+1 are queued before J matmuls of pair n:
    C2[m,b | i,mh] = sum_k U_i[k,m] d1[k,b]  (PE; moving operand is d1 itself)
    C3 = C2 * d2[m,b]                        (DVE psum-drain mult, no bcast)
    J[j,b | i]  = sum_mh W3h^T C3            (PE, W3 stationary, 128-part psum)
    one ACT drain per pair -> jbuf [128, (ii,b)] -> immediate per-pair DMA
  Output DRAM layout [128, 32, BS] = [(s,j), ii, b] so all 128 partitions
  carry DMA traffic; host reassembles to [b, j, i].
"""

import sys

sys.path.insert(0, "/opt/trn_rl_repo")

import numpy as np
import ml_dtypes
from contextlib import ExitStack

import concourse.bass as bass
import concourse.mybir as mybir
import concourse.tile as tile
from concourse import bacc
from concourse.bass_utils import run_bass_kernel_spmd

B, D, H = 8192, 64, 256
NCORES = 8
BS = B // NCORES  # 1024 batch per core
WN = 512  # batch window
NW = BS // WN  # 2 windows
NP = 16  # pairs per window (each pair covers 4 i-values)
UCH = 8  # u streaming chunks (8 i-values each)

BF = mybir.dt.bfloat16
F32 = mybir.dt.float32
MUL = mybir.AluOpType.mult
ADD = mybir.AluOpType.add
Tanh = mybir.ActivationFunctionType.Tanh
Square = mybir.ActivationFunctionType.Square
Copy = mybir.ActivationFunctionType.Copy

_CACHE = {}
TRACE = False


def _build():
    nc = bacc.Bacc("TRN2")
    xt_d = nc.dram_tensor("xt", [D, BS], BF, kind="ExternalInput")
    w1_d = nc.dram_tensor("w1", [D, H], BF, kind="ExternalInput")
    w2_d = nc.dram_tensor("w2", [H, H], BF, kind="ExternalInput")
    w3_d = nc.dram_tensor("w3", [H, D], BF, kind="ExternalInput")
    b1_d = nc.dram_tensor("b1", [H], F32, kind="ExternalInput")
    b2_d = nc.dram_tensor("b2", [H], F32, kind="ExternalInput")
    # u{kh}{mh}[k, i*128+m] = W1[i, kh*128+k] * W2[kh*128+k, mh*128+m]
    u_d = [
        [
            nc.dram_tensor(f"u{kh}{mh}", [128, D * 128], BF, kind="ExternalInput")
            for mh in range(2)
        ]
        for kh in range(2)
    ]
    # output layout [s*64+j, ii, b] with i = 4*(ii//2) + 2*s + ii%2;
    # host reassembles to [b, j, i]
    jac_d = nc.dram_tensor("jac", [128, D // 2, BS], F32, kind="ExternalOutput")

    with ExitStack() as ctx:
        tc = ctx.enter_context(tile.TileContext(nc))
        const = ctx.enter_context(tc.tile_pool(name="const", bufs=1))
        sb = ctx.enter_context(tc.tile_pool(name="sb", bufs=2))
        psf = ctx.enter_context(tc.tile_pool(name="psf", bufs=1, space="PSUM"))
        ps = ctx.enter_context(tc.tile_pool(name="ps", bufs=1, space="PSUM"))

        # ---- tiles ----
        w1_sb = const.tile([D, H], BF)
        w2_sb = [const.tile([128, H], BF, name=f"w2{k}") for k in range(2)]
        w3_sb = [const.tile([128, D], BF, name=f"w3{k}") for k in range(2)]
        b1_sb = const.tile([128, 2], F32)
        b2_sb = const.tile([128, 2], F32)
        u_sb = [
            [const.tile([128, D * 128], BF, name=f"u{kh}{mh}") for mh in range(2)]
            for kh in range(2)
        ]

        d1 = [[None, None] for _ in range(NW)]
        d2 = [[None, None] for _ in range(NW)]

        def fwd_window(w, xT):
            # layer 1: both tanh matmuls first, then the d1 (1-h^2) chain,
            # so layer 2's matmuls unblock as early as possible
            h1 = [
                sb.tile([128, WN], BF, tag=f"h1_{k}", name=f"h1_{k}") for k in range(2)
            ]
            for hh in range(2):
                a_ps = psf.tile([128, WN], F32, tag="fwd", name="a1_ps", bufs=2)
                nc.tensor.matmul(
                    a_ps, w1_sb[:, hh * 128 : (hh + 1) * 128], xT, start=True, stop=True
                )
                nc.scalar.activation(
                    out=h1[hh], in_=a_ps, func=Tanh, bias=b1_sb[:, hh : hh + 1]
                )

            for hh in range(2):
                sq = sb.tile([128, WN], F32, tag="sq", name="sq1")
                nc.scalar.activation(out=sq, in_=h1[hh], func=Square)
                d1[w][hh] = sb.tile(
                    [128, WN], BF, tag=f"d1_{w}_{hh}", name=f"d1_{w}_{hh}", bufs=1
                )
                nc.vector.tensor_scalar(
                    out=d1[w][hh], in0=sq, scalar1=-1.0, scalar2=1.0, op0=MUL, op1=ADD
                )

            for mh in range(2):
                a_ps = psf.tile([128, WN], F32, tag="fwd", name="a2_ps", bufs=2)
                for hh in range(2):
                    nc.tensor.matmul(
                        a_ps,
                        w2_sb[hh][:, mh * 128 : (mh + 1) * 128],
                        h1[hh],
                        start=(hh == 0),
                        stop=(hh == 1),
                    )
                h2 = sb.tile([128, WN], BF, tag="h2", name="h2")
                nc.scalar.activation(
                    out=h2, in_=a_ps, func=Tanh, bias=b2_sb[:, mh : mh + 1]
                )
                sq = sb.tile([128, WN], F32, tag="sq", name="sq2")
                nc.scalar.activation(out=sq, in_=h2, func=Square)
                d2[w][mh] = sb.tile(
                    [128, WN], BF, tag=f"d2_{w}_{mh}", name=f"d2_{w}_{mh}", bufs=1
                )
                nc.vector.tensor_scalar(
                    out=d2[w][mh], in0=sq, scalar1=-1.0, scalar2=1.0, op0=MUL, op1=ADD
                )

        # ---- DMA order: xT first, then small consts, then u chunks ----
        xTs = [sb.tile([D, WN], BF, tag="xT", name=f"xT{w}") for w in range(NW)]
        nc.sync.dma_start(out=xTs[0], in_=xt_d[:, 0:WN])
        nc.sync.dma_start(out=w1_sb, in_=w1_d[:, :])
        nc.sync.dma_start(out=xTs[1], in_=xt_d[:, WN : 2 * WN])
        for k in range(2):
            sl = slice(k * 128, (k + 1) * 128)
            nc.sync.dma_start(out=w2_sb[k], in_=w2_d[sl, :])
            nc.sync.dma_start(out=w3_sb[k], in_=w3_d[sl, :])
        nc.sync.dma_start(out=b1_sb, in_=b1_d.rearrange("(a p) -> p a", p=128))
        nc.sync.dma_start(out=b2_sb, in_=b2_d.rearrange("(a p) -> p a", p=128))
        csz = D * 128 // UCH
        for c in range(UCH):
            sl = slice(c * csz, (c + 1) * csz)
            for kh in range(2):
                for mh in range(2):
                    nc.sync.dma_start(out=u_sb[kh][mh][:, sl], in_=u_d[kh][mh][:, sl])

        # preload the ACT function table (1.3us) concurrently with the x DMA
        warm = sb.tile([1, 2], F32, tag="warm", name="warm", bufs=1)
        nc.scalar.activation(out=warm[:, 0:1], in_=b1_sb[0:1, 0:1], func=Tanh)

        fwd_window(0, xTs[0])

        # ---- pipelined pair loop ----
        TOT = NW * NP

        HW2 = WN // 2

        def emit_c2(n, split=False):
            w, t = divmod(n, NP)
            c3 = [[None, None], [None, None]]  # [s][mh]
            halves = ((0, WN),) if not split else ((0, HW2), (HW2, WN))
            for s in range(2):
                for mh in range(2):
                    c2_ps = ps.tile([128, 2 * WN], F32, tag="c2", name="c2_ps", bufs=2)
                    ct = sb.tile(
                        [128, 2 * WN], BF, tag=f"c3_{s}_{mh}", name=f"c3_{s}_{mh}"
                    )
                    for b0, b1 in halves:
                        for q in range(2):
                            i = 4 * t + 2 * s + q
                            for kh in range(2):
                                nc.tensor.matmul(
                                    c2_ps[:, q * WN + b0 : q * WN + b1],
                                    u_sb[kh][mh][:, i * 128 : (i + 1) * 128],
                                    d1[w][kh][:, b0:b1],
                                    start=(kh == 0),
                                    stop=(kh == 1),
                                )
                        nc.vector.tensor_tensor(
                            out=ct.rearrange("p (q b) -> p q b", q=2)[:, :, b0:b1],
                            in0=c2_ps.rearrange("p (q b) -> p q b", q=2)[:, :, b0:b1],
                            in1=d2[w][mh][:, None, b0:b1].broadcast_to(
                                [128, 2, b1 - b0]
                            ),
                            op=MUL,
                        )
                    c3[s][mh] = ct
            return c3

        def emit_j(n, c3, split=False):
            w, t = divmod(n, NP)
            j_ps = ps.tile([128, 2 * WN], F32, tag="jps", name="j_ps", bufs=1)
            jb = sb.tile([128, 2 * WN], F32, tag="jb", name="jb", bufs=2)
            wb = w * WN
            halves = ((0, WN),) if not split else ((0, HW2), (HW2, WN))
            for b0, b1 in halves:
                for s in range(2):
                    pview = j_ps[s * 64 : (s + 1) * 64, :]
                    for mh in range(2):
                        for q in range(2):
                            nc.tensor.matmul(
                                pview[:, q * WN + b0 : q * WN + b1],
                                w3_sb[mh],
                                c3[s][mh][:, q * WN + b0 : q * WN + b1],
                                start=(mh == 0),
                                stop=(mh == 1),
                            )
                nc.scalar.activation(
                    out=jb.rearrange("p (q b) -> p q b", q=2)[:, :, b0:b1],
                    in_=j_ps.rearrange("p (q b) -> p q b", q=2)[:, :, b0:b1],
                    func=Copy,
                )
                nc.sync.dma_start(
                    out=jac_d[:, 2 * t : 2 * t + 2, wb + b0 : wb + b1],
                    in_=jb.rearrange("p (q b) -> p q b", q=2)[:, :, b0:b1],
                )

        prev = None
        for n in range(TOT):
            c3 = emit_c2(n, split=(n == TOT - 1))
            if n == 2:
                fwd_window(1, xTs[1])
            if prev is not None:
                emit_j(*prev)
            prev = (n, c3)
        emit_j(prev[0], prev[1], split=True)
    nc.compile()
    return nc


def kernel(x, W1, b1, W2, b2, W3, b3):
    x = np.asarray(x, dtype=np.float32)
    bf = ml_dtypes.bfloat16
    if "nc" not in _CACHE:
        _CACHE["nc"] = _build()
    nc = _CACHE["nc"]

    W1f = np.asarray(W1, np.float32)
    W2f = np.asarray(W2, np.float32)
    Ufull = W1f[:, :, None] * W2f[None, :, :]  # [i, k, m]
    shared = {
        "w1": W1f.astype(bf),
        "w2": W2f.astype(bf),
        "w3": np.asarray(W3, np.float32).astype(bf),
        "b1": np.asarray(b1, np.float32),
        "b2": np.asarray(b2, np.float32),
    }
    for kh in range(2):
        for mh in range(2):
            u = Ufull[:, kh * 128 : (kh + 1) * 128, mh * 128 : (mh + 1) * 128]
            u = np.ascontiguousarray(u.transpose(1, 0, 2).reshape(128, D * 128))
            shared[f"u{kh}{mh}"] = u.astype(bf)
    xt = np.ascontiguousarray(x.T.astype(bf))  # [D, B]
    in_maps = [
        {"xt": np.ascontiguousarray(xt[:, c * BS : (c + 1) * BS]), **shared}
        for c in range(NCORES)
    ]
    res = run_bass_kernel_spmd(nc, in_maps, core_ids=list(range(NCORES)), trace=TRACE)
    _CACHE["last_res"] = res
    out = np.empty((B, D, D), np.float32)
    for c in range(NCORES):
        # jac[s*64+j, 2t+q, b] = J[j, b, 4t+2s+q]
        arr = res.results[c]["jac"].reshape(2, 64, 16, 2, BS)
        out[c * BS : (c + 1) * BS] = (
            arr.transpose(4, 1, 2, 0, 3).reshape(BS, D, D)
        )
    return out
